# revision 7
# baseline (speedup 1.0000x reference)
"""Bahdanau attention cell (location-sensitive) on 8 TRN2 NeuronCores.

Sharding: data-parallel over the batch dim (64 -> 8 batches/core); all
params (conv kernel, location dense, score v/b) are tiny and replicated.

v2 design ([t-on-partitions, a-on-free] layout):
  1. The location conv is folded away on the host: loc[t,a] =
     sum_k prev[t+k-15] * M[k,a] with M = conv_w @ loc_w ([31, 256]).
     The PE contracts im2col(prev) [64, 128] stationaries (31 taps for
     chunk ch, 31 taps for ch+1, 2 ones rows) against a [64, 512] moving
     [M ; q_hi ; q_lo] -> PSUM[toff, (ch a | ch+1 a)] = loc + q.
  2. w_memory stays fp32 (float32r view: 1 col/cycle on PE for >=256-col
     moving) -> no cast DMA; w streams on all three DMA queues (sync +
     scalar HW DGE, gpsimd SW DGE) in [128, 1024] group tiles.
     ident16 @ w accumulates w on top of loc+q in PSUM (start=False).
  3. tanh on ACT -> fp16; fused multiply-by-v + reduce via DVE
     tensor_tensor_reduce (fp16 operands -> 2x mode; fp32 [128,1] accum)
     -> e_cols[toff, b*16+ch].
  4. softmax in the column domain (as before): tiny ops, gpsimd
     partition_all_reduce for cross-partition max/sum, one PE transpose,
     stream-matched DMAs to the padded outputs.
"""

import sys

sys.path.insert(0, "/opt/trn_rl_repo")

import numpy as np

import concourse.bacc as bacc
import concourse.bass as bass
import concourse.bass_isa as bass_isa
import concourse.tile as tile
from concourse import mybir
from concourse.bass_utils import run_bass_kernel_spmd

B, T, A, F, KW = 64, 2000, 256, 32, 31
NCORES = 8
BL = B // NCORES  # 8 batches per core
PAD = (KW - 1) // 2  # 15
TC = 2048  # t padded to 16 chunks of 128
TP2 = TC + 2 * PAD  # padded prev length
NCH = TC // 128  # 16 chunks per batch
F32 = mybir.dt.float32
F32R = mybir.dt.float32r
F16 = mybir.dt.float16
BF16 = mybir.dt.bfloat16

# batch -> DMA queue for its w tiles: 's'=sync(SP), 'a'=scalar(ACT HW DGE),
# 'g'=gpsimd (SW DGE). Balance so each queue carries a similar byte load.
WQ = ["s", "s", "s", "a", "a", "g", "g", "g"]


def build_program():
    nc = bacc.Bacc("TRN2", target_bir_lowering=False)

    wmem = nc.dram_tensor("wmem", [BL, TC, A], F32R, kind="ExternalInput")
    prevp = nc.dram_tensor("prevp", [BL, TP2], F32, kind="ExternalInput")
    ones2 = nc.dram_tensor("ones2", [2, TC], BF16, kind="ExternalInput")
    mq = nc.dram_tensor("mq", [64, BL * 512], BF16, kind="ExternalInput")
    vrep = nc.dram_tensor("vrep", [128, 4 * A], F16, kind="ExternalInput")
    identb = nc.dram_tensor("identb", [128, 128], F32R, kind="ExternalInput")
    ident32 = nc.dram_tensor("ident32", [128, 128], F32, kind="ExternalInput")
    maskc = nc.dram_tensor("maskc", [128, 128], F32, kind="ExternalInput")
    prevc = nc.dram_tensor("prevc", [128, 128], F32, kind="ExternalInput")
    out_w = nc.dram_tensor("out_w", [BL, TC], F32, kind="ExternalOutput")
    out_nw = nc.dram_tensor("out_nw", [BL, TC], F32, kind="ExternalOutput")

    with tile.TileContext(nc) as tc:
        with (
            tc.tile_pool(name="singles", bufs=1) as singles,
            tc.tile_pool(name="wg", bufs=20) as wgpool,
            tc.tile_pool(name="th", bufs=4) as thpool,
            tc.tile_pool(name="y", bufs=2) as scrpool,
            tc.tile_pool(name="sm", bufs=1) as spool,
            tc.tile_pool(name="pz", bufs=4, space="PSUM") as pzpool,
        ):
            # ---- constants on the scalar(ACT) HW queue ----
            identr = singles.tile([128, 128], F32R, tag="identr")
            nc.scalar.dma_start(out=identr[:], in_=identb[:])
            mq_sb = singles.tile([64, BL * 512], BF16, tag="mq")
            nc.scalar.dma_start(out=mq_sb[:], in_=mq[:])
            vrep_sb = singles.tile([128, 4 * A], F16, tag="vrep")
            nc.scalar.dma_start(out=vrep_sb[:], in_=vrep[:])
            id32_sb = singles.tile([128, 128], F32, tag="ident32")
            nc.scalar.dma_start(out=id32_sb[:], in_=ident32[:])
            maskc_sb = singles.tile([128, 128], F32, tag="maskc")
            nc.scalar.dma_start(out=maskc_sb[:], in_=maskc[:])
            prevc_sb = singles.tile([128, 128], F32, tag="prevc")
            nc.scalar.dma_start(out=prevc_sb[:], in_=prevc[:])

            # ---- im2col stationaries (gpsimd cast-DMA fp32->bf16) ----
            # rows 0..30: prevp[b, t+k] (chunk ch taps)
            # rows 31..61: prevp[b, t+128+k] (chunk ch+1 taps; cols < TC-128)
            # rows 62..63: 1.0 (q_hi / q_lo fold rows)
            im_sb = []
            for b in range(BL):
                im = singles.tile([64, TC], BF16, tag=f"im{b}")
                base = prevp[b, :]
                nc.gpsimd.dma_start(
                    out=im[0:31, :],
                    in_=bass.AP(
                        tensor=base.tensor,
                        offset=base.offset,
                        ap=[[1, 31], [1, TC]],
                    ),
                )
                nc.gpsimd.dma_start(
                    out=im[31:62, 0 : TC - 128],
                    in_=bass.AP(
                        tensor=base.tensor,
                        offset=base.offset + 128,
                        ap=[[1, 31], [1, TC - 128]],
                    ),
                )
                nc.gpsimd.dma_start(out=im[62:64, :], in_=ones2[:])
                im_sb.append(im)

            # ---- w group tiles [128, 1024] fp32 (float32r view) ----
            qeng = {"s": nc.sync, "a": nc.scalar, "g": nc.gpsimd}
            wtiles = {}
            for b in range(BL):
                base = wmem[b, :, :]
                for g in range(4):
                    wt = wgpool.tile([128, 1024], F32R, tag="wg")
                    qeng[WQ[b]].dma_start(
                        out=wt[:],
                        in_=bass.AP(
                            tensor=base.tensor,
                            offset=base.offset + (4 * g) * 128 * A,
                            ap=[[A, 128], [128 * A, 4], [1, A]],
                        ),
                    )
                    wtiles[(b, g)] = wt

            # ---- main pass ----
            e_cols = spool.tile([128, 128], F32, tag="e_cols")
            for b in range(BL):
                im = im_sb[b]
                mv = mq_sb[:, b * 512 : (b + 1) * 512]
                for g in range(4):
                    wt = wtiles[(b, g)]
                    z = pzpool.tile([128, 1024], F32, tag="z")
                    for h in range(2):
                        c0 = 4 * g + 2 * h
                        nc.tensor.matmul(
                            z[:, h * 512 : (h + 1) * 512],
                            im[:, c0 * 128 : (c0 + 1) * 128],
                            mv,
                            start=True,
                            stop=False,
                        )
                    for h in range(2):
                        nc.tensor.matmul(
                            z[:, h * 512 : (h + 1) * 512],
                            identr[:],
                            wt[:, h * 512 : (h + 1) * 512],
                            start=False,
                            stop=True,
                        )
                    th = thpool.tile([128, 1024], F16, tag="th")
                    nc.scalar.activation(
                        out=th[:],
                        in_=z[:],
                        func=mybir.ActivationFunctionType.Tanh,
                    )
                    y = scrpool.tile([128, 1024], F16, tag="y")
                    nc.vector.tensor_mul(y[:], th[:], vrep_sb[:])
                    for j in range(4):
                        col = b * NCH + 4 * g + j
                        # [128, 256] -> [128, 1] per chunk: scalar fp32 out
                        # keeps the DVE in 2x mode (fp16 unit-stride input)
                        nc.vector.tensor_reduce(
                            out=e_cols[:, col : col + 1],
                            in_=y[:, j * A : (j + 1) * A],
                            axis=mybir.AxisListType.X,
                            op=mybir.AluOpType.add,
                        )

            # ---- masked softmax in the column domain [toff, b*16+j] ----
            msk = spool.tile([128, 128], F32, tag="msk")
            nc.vector.tensor_mul(msk[:], e_cols[:], maskc_sb[:])
            m1 = spool.tile([128, 8], F32, tag="m1")
            nc.vector.tensor_reduce(
                out=m1[:],
                in_=msk[:].rearrange("p (b j) -> p b j", b=8),
                axis=mybir.AxisListType.X,
                op=mybir.AluOpType.max,
            )
            mx = spool.tile([128, 8], F32, tag="mx")
            nc.gpsimd.partition_all_reduce(
                mx[:], m1[:], 128, bass_isa.ReduceOp.max
            )
            mxr = spool.tile([128, 128], F32, tag="mxr")
            nc.vector.tensor_copy(
                out=mxr[:].rearrange("p (b j) -> p b j", b=8),
                in_=bass.AP(
                    tensor=mx.tensor,
                    offset=mx.offset,
                    ap=[[8, 128], [1, 8], [0, 16]],
                ),
            )
            sub = spool.tile([128, 128], F32, tag="sub")
            nc.vector.tensor_sub(sub[:], e_cols[:], mxr[:])
            ex = spool.tile([128, 128], F32, tag="ex")
            nc.scalar.activation(
                out=ex[:], in_=sub[:], func=mybir.ActivationFunctionType.Exp
            )
            num = spool.tile([128, 128], F32, tag="num")
            nc.vector.tensor_mul(num[:], ex[:], maskc_sb[:])
            s1 = spool.tile([128, 8], F32, tag="s1")
            nc.vector.tensor_reduce(
                out=s1[:],
                in_=num[:].rearrange("p (b j) -> p b j", b=8),
                axis=mybir.AxisListType.X,
                op=mybir.AluOpType.add,
            )
            ss = spool.tile([128, 8], F32, tag="ss")
            nc.gpsimd.partition_all_reduce(
                ss[:], s1[:], 128, bass_isa.ReduceOp.add
            )
            riv = spool.tile([128, 8], F32, tag="riv")
            nc.vector.reciprocal(riv[:], ss[:])
            rivr = spool.tile([128, 128], F32, tag="rivr")
            nc.vector.tensor_copy(
                out=rivr[:].rearrange("p (b j) -> p b j", b=8),
                in_=bass.AP(
                    tensor=riv.tensor,
                    offset=riv.offset,
                    ap=[[8, 128], [1, 8], [0, 16]],
                ),
            )
            ow_c = spool.tile([128, 128], F32, tag="ow_c")
            nc.vector.tensor_mul(ow_c[:], num[:], rivr[:])
            # transpose to [(b,j)-partitions, toff-free] and stream out
            pe_t = pzpool.tile([128, 1024], F32, tag="z")
            nc.tensor.matmul(
                pe_t[:, 0:128], ow_c[:], id32_sb[:], is_transpose=True,
                start=True, stop=True,
            )
            ow_t = spool.tile([128, 128], F32, tag="ow_t")
            nc.vector.tensor_copy(out=ow_t[:], in_=pe_t[:, 0:128])
            nw_t = spool.tile([128, 128], F32, tag="nw_t")
            nc.vector.tensor_add(nw_t[:], ow_t[:], prevc_sb[:])
            nc.sync.dma_start(out=out_w[:], in_=ow_t[:])
            nc.sync.dma_start(out=out_nw[:], in_=nw_t[:])

    nc.finalize()
    return nc


def make_in_maps(query, prev_weights, w_memory, memory_lengths, conv_w, conv_b,
                 loc_w, score_v, score_b):
    """Host-side prep (small params only) + batch sharding."""
    import ml_dtypes

    query = np.asarray(query, np.float32)
    prev_weights = np.asarray(prev_weights, np.float32)
    w_memory = np.asarray(w_memory, np.float32)
    memory_lengths = np.asarray(memory_lengths)
    conv_w = np.asarray(conv_w, np.float32)
    conv_b = np.asarray(conv_b, np.float32)
    loc_w = np.asarray(loc_w, np.float32)
    score_v = np.asarray(score_v, np.float32)
    score_b = np.asarray(score_b, np.float32)

    # conv folded into the location dense: loc = im2col(prev) @ M + q_eff
    M = conv_w[:, 0, :] @ loc_w  # [31, 256]
    q_eff = query + score_b[None, :] + (conv_b @ loc_w)[None, :]  # [B, A]

    def _bf16(x):
        u = x.astype(np.float32).view(np.uint32)
        u = (u + 0x8000 + ((u >> 16) & 1)) & 0xFFFF0000
        return u.view(np.float32)

    vrep = np.tile(score_v[None, :], (128, 4)).astype(np.float16)
    identb = np.eye(128, dtype=np.float32)
    ident32 = np.eye(128, dtype=np.float32)
    ones2 = np.ones((2, TC), np.float32).astype(ml_dtypes.bfloat16)
    prevp_full = np.pad(prev_weights, ((0, 0), (PAD, TC - T + PAD)))

    # column-domain t index: t = j*128 + toff
    tt = np.arange(NCH)[None, :] * 128 + np.arange(128)[:, None]  # [toff, j]

    in_maps = []
    for i in range(NCORES):
        s = slice(i * BL, (i + 1) * BL)
        wshard = np.zeros((BL, TC, A), np.float32)
        wshard[:, :T, :] = w_memory[s]
        # moving operand [64, 512] per batch: rows 0..30 = M (chunk ch
        # cols), rows 31..61 = M (chunk ch+1 cols), rows 62/63 = q split
        # into two bf16-exact rows (hi + residual) across all 512 cols.
        mq_c = np.zeros((64, BL * 512), np.float32)
        for b in range(BL):
            qv = q_eff[i * BL + b]
            q_hi = _bf16(qv)
            q_lo = _bf16(qv - q_hi)
            cs0 = slice(b * 512, b * 512 + A)
            cs1 = slice(b * 512 + A, b * 512 + 2 * A)
            mq_c[0:KW, cs0] = M
            mq_c[31 : 31 + KW, cs1] = M
            mq_c[62, b * 512 : (b + 1) * 512] = np.concatenate([q_hi, q_hi])
            mq_c[63, b * 512 : (b + 1) * 512] = np.concatenate([q_lo, q_lo])
        lens = memory_lengths[s]
        mask_c = np.zeros((128, 128), np.float32)
        prev_c = np.zeros((128, 128), np.float32)
        for b in range(BL):
            mask_c[:, b * NCH : (b + 1) * NCH] = (
                (tt < T) & (tt < lens[b])
            ).astype(np.float32)
            # prevc is consumed AFTER the transpose: [(b,j)-rows, toff]
            padded = np.pad(prev_weights[i * BL + b], (0, TC - T))
            prev_c[b * NCH : (b + 1) * NCH, :] = padded.reshape(NCH, 128)
        in_maps.append(
            {
                "wmem": wshard,
                "prevp": np.ascontiguousarray(prevp_full[s]),
                "ones2": ones2,
                "mq": mq_c.astype(ml_dtypes.bfloat16),
                "vrep": vrep,
                "identb": identb,
                "ident32": ident32,
                "maskc": mask_c,
                "prevc": prev_c,
            }
        )
    return in_maps


_NC_CACHE = {}


def _get_nc():
    if "nc" not in _NC_CACHE:
        _NC_CACHE["nc"] = build_program()
    return _NC_CACHE["nc"]


def run(inputs, trace=False, tmpdir=None):
    """Run on 8 NeuronCores; returns ((output, new_weights), BassKernelResults)."""
    nc = _get_nc()
    in_maps = make_in_maps(**inputs)
    res = run_bass_kernel_spmd(
        nc, in_maps, core_ids=list(range(NCORES)), trace=trace, tmpdir=tmpdir
    )
    output = np.concatenate(
        [res.results[i]["out_w"][:, :T] for i in range(NCORES)], axis=0
    )
    new_w = np.concatenate(
        [res.results[i]["out_nw"][:, :T] for i in range(NCORES)], axis=0
    )
    return (output.astype(np.float32), new_w.astype(np.float32)), res


def kernel(**inputs):
    (output, new_w), _ = run(inputs, trace=False)
    return output, new_w
